# revision 8
# baseline (speedup 1.0000x reference)
"""Trainium2 Bass kernel for nn_Attention pooling module (s-major redesign).

Math (per batch b, using sum(attn)==1 and sigmoid(x)=0.5*(1+tanh(x/2))):
    zT[s, e]  = sum_d seq[b, s, d] * w1[e, d] + bias_b[e]
                    where bias_b = w1_bias + w2(tgt_b) + w2_bias   (sigmoid input)
    T[s]      = sum_e tanh(zT[s, e] / 2)           (scores = 48 + T/2)
    E[s]      = mask[b, s] ? 0 : exp(T[s]/2 - 48)  (unnormalized attn)
    P[e], Z   = sum_s E[s] * [zT[s, e] | 1]
    out[b, e] = P[e]/Z - corr_b[e]                 where corr_b = w2(tgt_b) + w2_bias

Device layout (per core, 256 batches, all-data bf16):
  - seq uploaded ONCE, d-major with a ones row: [32 groups, 97, 8*512]
  - proj: per 128-token chunk, matmul(lhsT=seq chunk [97,128] stationary,
    rhs=[w1.T ; bias_b] [97, 96]) -> PSUM zT [128s, 96e].  Unit = 4 batches
    in one 4-bank PSUM tile (batches at 512-col offsets).
  - tanh on ScalarE per unit [128, 4x384] (exp_and_others table set only,
    no table switches); free-dim reduce on DVE -> per-chunk score sums.
  - exp per 16 batches [128, 64] with scale=0.5, mask folded in as
    {-1e9, -96} additive bias.
  - pool: per (batch, chunk) matmul(lhsT=E chunk [128, 1], rhs=zT chunk
    +ones col [128, 97]) accumulating into PSUM spare cols, row q.
  - final: P/Z - corr on DVE, fp32 out.
"""

from contextlib import ExitStack

import numpy as np
import ml_dtypes

import concourse.bass as bass
import concourse.bacc as bacc
import concourse.tile as tile
from concourse import mybir
from concourse.bass_utils import run_bass_kernel_spmd

BF16 = mybir.dt.bfloat16
F32 = mybir.dt.float32
NP_BF16 = ml_dtypes.bfloat16

N_CORES = 8
B = 2048
S = 512
D = 96
BC = B // N_CORES      # 256 batches per core
GROUP = 8              # batches per seqt DMA group
NGROUP = BC // GROUP   # 32
UNIT = 4               # batches per PSUM unit (4 banks)
NUNIT = BC // UNIT     # 64
NCHUNK = S // 128      # 4 token chunks of 128
PANEL = 128            # output panel
NPANEL = BC // PANEL   # 2
UPP = PANEL // UNIT    # units per panel = 32
ZT_N = 6               # persistent ztsb staging tiles
DP = D + 1             # 97: zT chunk + ones col

Tanh = mybir.ActivationFunctionType.Tanh
Exp = mybir.ActivationFunctionType.Exp
AX = mybir.AxisListType.X


def build_program() -> bass.Bass:
    nc = bacc.Bacc(
        "TRN2", target_bir_lowering=False, debug=False, num_devices=N_CORES
    )

    seqt_d = nc.dram_tensor(
        "seqt", [NGROUP, D + 1, GROUP * S], BF16, kind="ExternalInput"
    )
    w1rep_d = nc.dram_tensor("w1rep", [D, GROUP * D], BF16, kind="ExternalInput")
    brow_d = nc.dram_tensor("brow", [NGROUP, GROUP * D], BF16, kind="ExternalInput")
    maskm_d = nc.dram_tensor("maskm", [NUNIT // 4, 128, 64], F32, kind="ExternalInput")
    corr_d = nc.dram_tensor("corr", [BC, D], F32, kind="ExternalInput")
    out_d = nc.dram_tensor("out", [BC, D], F32, kind="ExternalOutput")

    with tile.TileContext(nc) as tc, ExitStack() as ctx:
        const_pool = ctx.enter_context(tc.tile_pool(name="const", bufs=1))
        seqp = ctx.enter_context(tc.tile_pool(name="seqp", bufs=3))
        sigp = ctx.enter_context(tc.tile_pool(name="sigp", bufs=3))
        smp = ctx.enter_context(tc.tile_pool(name="smp", bufs=2))
        zpsum = ctx.enter_context(
            tc.tile_pool(name="zpsum", bufs=2, space=bass.MemorySpace.PSUM)
        )

        # persistent rhs tiles: [w1.T replicated 8x ; per-group bias row]
        rhs_tiles = []
        for k in (0, 1):
            t = const_pool.tile([D + 1, GROUP * D], BF16, name=f"rhs{k}", tag=f"rhs{k}")
            nc.scalar.dma_start(t[0:D, :], w1rep_d[:])
            rhs_tiles.append(t)
        # persistent zT staging tiles (bf16, 4 batches x 4 chunks x 97 cols);
        # ones cols written once, data cols rewritten per unit
        zt_tiles = []
        for k in range(ZT_N):
            t = const_pool.tile(
                [128, UNIT * NCHUNK * DP], BF16, name=f"ztsb{k}", tag=f"ztsb{k}"
            )
            ones_ap = t[:].rearrange("p (q j e) -> p q j e", q=UNIT, j=NCHUNK, e=DP)[
                :, :, :, D : D + 1
            ]
            nc.vector.memset(ones_ap, 1.0)
            zt_tiles.append(t)
        # persistent zero-padded exp tiles: E(u4, q, j) at col u4*64 + q*17 + j*4,
        # zeros elsewhere; [128, base+q] slices give a [128, 4] pool lhsT whose
        # only nonzero column is q (so rows != q accumulate exact zeros)
        exz_tiles = []
        for k in (0, 1):
            t = const_pool.tile([128, 256], BF16, name=f"exz{k}", tag=f"exz{k}")
            nc.vector.memset(t[:], 0.0)
            exz_tiles.append(t)

        group_sb = {}     # g -> (seqt_sb, rhs tile)
        tg_tiles = {}     # gg -> (scores [128,64] f32, mask [128,64] f32)
        exp_tiles = {}    # gg -> E [128,64] bf16
        unit_zt = {}      # u -> ztsb AP
        panel_pool = {}   # panel -> pooled [128, 97] f32
        corr_sb = {}      # panel -> corr tile

        for v in range(NUNIT + 4):
            # -------- produce stage for unit v --------
            zt_ps = zpsum.tile([128, 2048], F32, name="zt_ps")
            if v < NUNIT:
                u = v
                g, ug = divmod(u, 2)
                if ug == 0:
                    seqt_sb = seqp.tile([D + 1, GROUP * S], BF16, name="seqt_sb")
                    nc.sync.dma_start(seqt_sb[:], seqt_d[g])
                    rhsg = rhs_tiles[g % 2]
                    nc.scalar.dma_start(
                        rhsg[D : D + 1, :], brow_d[g : g + 1, :]
                    )
                    group_sb[g] = (seqt_sb, rhsg)
                seqt_sb, rhsg = group_sb[g]
                panel = u // UPP
                if u % UPP == 0:
                    pooled = smp.tile([128, DP], F32, name="pooled", tag="pooled")
                    panel_pool[panel] = pooled
                    csb = smp.tile([128, D], F32, name="csb", tag="csb")
                    nc.scalar.dma_start(
                        csb[:], corr_d[panel * PANEL : (panel + 1) * PANEL, :]
                    )
                    corr_sb[panel] = csb

                gg, u4 = divmod(u, 4)
                if u4 == 0:
                    sc2g = smp.tile([128, 64], F32, name="sc2g", tag="sc")
                    mk2g = smp.tile([128, 64], F32, name="mk2g", tag="mk")
                    nc.scalar.dma_start(mk2g[:], maskm_d[gg])
                    tg_tiles[gg] = (sc2g, mk2g)
                sc2g, mk2g = tg_tiles[gg]

                # 16 projection matmuls -> zT in PSUM
                for q in range(UNIT):
                    i = ug * UNIT + q      # batch within group
                    for j in range(NCHUNK):
                        nc.tensor.matmul(
                            zt_ps[:, q * 512 + j * D : q * 512 + (j + 1) * D],
                            seqt_sb[:, i * S + j * 128 : i * S + (j + 1) * 128],
                            rhsg[:, i * D : (i + 1) * D],
                            start=True,
                            stop=True,
                        )
                zt3 = zt_ps[:].rearrange("p (q c) -> p q c", q=UNIT)[
                    :, :, 0 : NCHUNK * D
                ]
                th = sigp.tile([128, UNIT * NCHUNK * D], BF16, name="th")
                nc.scalar.activation(
                    th[:].rearrange("p (q c) -> p q c", q=UNIT), zt3, Tanh, scale=0.5
                )
                # per-chunk score sums -> sc2g cols [u4*16, u4*16+16)
                nc.vector.reduce_sum(
                    sc2g[:, u4 * 16 : (u4 + 1) * 16],
                    th[:].rearrange("p (q j e) -> p q j e", q=UNIT, j=NCHUNK, e=D),
                    axis=AX,
                )
                # zT copy PSUM -> bf16 staging (97-col stride, ones col kept)
                ztsb = zt_tiles[u % ZT_N]
                unit_zt[u] = ztsb
                nc.vector.tensor_copy(
                    ztsb[:].rearrange(
                        "p (q j e) -> p q j e", q=UNIT, j=NCHUNK, e=DP
                    )[:, :, :, 0:D],
                    zt3.rearrange("p q (j e) -> p q j e", j=NCHUNK),
                )
                if u4 == 3:
                    sm2g = smp.tile([128, 64], F32, name="sm2g", tag="sm")
                    nc.vector.tensor_add(sm2g[:], sc2g[:], mk2g[:])
                    exz = exz_tiles[gg % 2]
                    exz_out = bass.AP(
                        tensor=exz.tensor,
                        offset=exz.offset,
                        ap=[[256, 128], [64, 4], [17, 4], [4, 4]],
                    )
                    nc.scalar.activation(exz_out, sm2g[:], Exp, scale=0.5)
                    exp_tiles[gg] = exz

            # -------- pool stage for unit v-4 --------
            if v >= 4:
                u = v - 4
                gg, u4 = divmod(u, 4)
                exz = exp_tiles[gg]
                ztsb = unit_zt.pop(u)
                for j in range(NCHUNK):
                    for q in range(UNIT):
                        base = u4 * 64 + q * 16 + j * 4
                        nc.tensor.matmul(
                            zt_ps[0:UNIT, 384 : 384 + DP],
                            exz[:, base : base + UNIT],
                            ztsb[:, (q * NCHUNK + j) * DP : (q * NCHUNK + j + 1) * DP],
                            start=(j == 0 and q == 0),
                            stop=(j == NCHUNK - 1 and q == UNIT - 1),
                        )
                panel = u // UPP
                pooled = panel_pool[panel]
                pu = u % UPP
                pstg = smp.tile([UNIT, DP], F32, name="pstg", tag="pstg", bufs=3)
                nc.vector.tensor_copy(pstg[:], zt_ps[0:UNIT, 384 : 384 + DP])
                nc.scalar.dma_start(
                    pooled[pu * UNIT : (pu + 1) * UNIT, 0:DP], pstg[:]
                )
                # -------- panel finalize --------
                if pu == UPP - 1:
                    rz = smp.tile([128, 1], F32, name="rz", tag="rz")
                    nc.vector.reciprocal(rz[:], pooled[:, D : D + 1])
                    osb = smp.tile([128, D], F32, name="osb", tag="osb")
                    nc.vector.tensor_scalar_mul(osb[:], pooled[:, 0:D], rz[:, 0:1])
                    ofb = smp.tile([128, D], F32, name="ofb", tag="ofb")
                    nc.vector.tensor_sub(ofb[:], osb[:], corr_sb[panel][:])
                    nc.scalar.dma_start(
                        out_d[panel * PANEL : (panel + 1) * PANEL, :], ofb[:]
                    )

    nc.compile()
    return nc


def prepare_in_maps(inputs: dict) -> list[dict]:
    seq = np.asarray(inputs["seq_item_embedding"], dtype=np.float32)
    tgt = np.asarray(inputs["target_item_embedding"], dtype=np.float32)
    mask = np.asarray(inputs["mask"])
    w1w = np.asarray(inputs["w1_weight"], dtype=np.float32)
    w1b = np.asarray(inputs["w1_bias"], dtype=np.float32)
    w2w = np.asarray(inputs["w2_weight"], dtype=np.float32)
    w2b = np.asarray(inputs["w2_bias"], dtype=np.float32)

    seq_bf = seq.astype(NP_BF16)
    corr_all = (tgt[:, 0, :] @ w2w.T + w2b).astype(np.float32)   # [B, D]
    bias_all = (corr_all + w1b).astype(np.float32)               # [B, D]
    m = mask[:, :S, 0]                                           # [B, S] bool

    w1rep = np.tile(np.ascontiguousarray(w1w.T), (1, GROUP)).astype(NP_BF16)

    in_maps = []
    for c in range(N_CORES):
        sl = slice(c * BC, (c + 1) * BC)
        sc = seq_bf[sl]  # [BC, S, D]
        seqt_core = np.ascontiguousarray(
            sc.reshape(NGROUP, GROUP, S, D).transpose(0, 3, 1, 2)
        ).reshape(NGROUP, D, GROUP * S)
        seqt = np.concatenate(
            [seqt_core, np.ones((NGROUP, 1, GROUP * S), dtype=NP_BF16)], axis=1
        )
        # maskm[gg, p, b16*4 + j] = mask[b, j*128+p] ? -1e9 : -96
        mc = m[sl].reshape(NUNIT // 4, 16, NCHUNK, 128)
        maskm = np.where(mc, np.float32(-1e9), np.float32(-96.0)).transpose(
            0, 3, 1, 2
        ).reshape(NUNIT // 4, 128, 64)
        in_maps.append(
            {
                "seqt": np.ascontiguousarray(seqt),
                "w1rep": w1rep,
                "brow": np.ascontiguousarray(
                    bias_all[sl].astype(NP_BF16).reshape(NGROUP, GROUP * D)
                ),
                "maskm": np.ascontiguousarray(maskm),
                "corr": np.ascontiguousarray(corr_all[sl]),
            }
        )
    return in_maps


_CACHED_NC = None


def run(inputs: dict, trace: bool = False, tmpdir: str | None = None):
    global _CACHED_NC
    in_maps = prepare_in_maps(inputs)
    if _CACHED_NC is None:
        _CACHED_NC = build_program()
    res = run_bass_kernel_spmd(
        _CACHED_NC, in_maps, list(range(N_CORES)), trace=trace, tmpdir=tmpdir
    )
    out = np.concatenate([r["out"] for r in res.results], axis=0)
    return out, res


def kernel(**inputs) -> np.ndarray:
    out, _ = run(inputs, trace=False)
    return out


# revision 9
# speedup vs baseline: 4.3647x; 4.3647x over previous
"""Trainium2 Bass kernel for nn_Attention pooling module (s-major, dual upload).

Math (per batch b, using sum(attn)==1 and sigmoid(x)=0.5*(1+tanh(x/2))):
    zT[s, e]  = sum_d seq[b, s, d] * w1[e, d] + bias_b[e]
                    where bias_b = w1_bias + w2(tgt_b) + w2_bias   (sigmoid input)
    T[s]      = sum_e tanh(zT[s, e] / 2)           (scores = 48 + T/2)
    E[s]      = mask[b, s] ? 0 : exp(T[s]/2 - 48)  (unnormalized attn)
    P[e], Z   = sum_s E[s] * [seq[s, e] | 1]       (pool RAW seq + ones col)
    out[b, :] = (P/Z) @ w1.T + w1_bias             (re-projection, sum(attn)==1)

Device layout (per core, 256 batches, data bf16):
  - seq uploaded twice: d-major [32, 96, 8*512] for the projection
    (96 rows = 12 SDMA engines; the ones row for the bias contraction is
    memset on-chip), and s-major [32, 128, 8*4*97] for pooling (full 16
    engines; col 96 of each chunk is 1.0 so pooling also yields Z).
  - proj: per 128-token chunk, matmul(lhsT=seq chunk [97,128] stationary,
    rhs=[w1.T ; bias_b] [97, 96]) -> PSUM zT [128s, 96e].  Unit = 4 batches
    in one 4-bank PSUM tile (batches at 512-col offsets).
  - tanh on ScalarE per unit [128, 4x384] with scale=0.5 (exp_and_others
    set only -> no activation-table switches); scores via a 2x-accelerated
    bf16 add-tree + one 1x reduce on DVE.
  - exp per 16 batches [128, 64] with scale=0.5, mask folded in as
    {-1e9, -96} additive bias, written zero-padded (stride 17) so a
    [128, 4] slice is a pool lhsT whose only nonzero column is q.
  - pool: 16 matmuls accumulate [4, 97] (P|Z) into PSUM spare cols; DVE
    copy -> SBUF; SBUF->SBUF DMA gathers units into the [128, 97] panel.
  - final per panel: P/Z, PE transpose, re-projection with bias row.
"""

from contextlib import ExitStack

import numpy as np
import ml_dtypes

import concourse.bass as bass
import concourse.bacc as bacc
import concourse.tile as tile
from concourse import mybir
from concourse.bass_utils import run_bass_kernel_spmd

BF16 = mybir.dt.bfloat16
F32 = mybir.dt.float32
NP_BF16 = ml_dtypes.bfloat16

N_CORES = 8
B = 2048
S = 512
D = 96
BC = B // N_CORES      # 256 batches per core
GROUP = 8              # batches per DMA group
NGROUP = BC // GROUP   # 32
UNIT = 4               # batches per PSUM unit (4 banks)
NUNIT = BC // UNIT     # 64
NCHUNK = S // 128      # 4 token chunks of 128
PANEL = 128            # output panel
NPANEL = BC // PANEL   # 2
UPP = PANEL // UNIT    # units per panel = 32
DP = D + 1             # 97: seq chunk + ones col

Tanh = mybir.ActivationFunctionType.Tanh
Exp = mybir.ActivationFunctionType.Exp
AX = mybir.AxisListType.X


def build_program() -> bass.Bass:
    nc = bacc.Bacc(
        "TRN2", target_bir_lowering=False, debug=False, num_devices=N_CORES
    )

    seqt_d = nc.dram_tensor(
        "seqt", [NGROUP, D, GROUP * S], BF16, kind="ExternalInput"
    )
    natg_d = nc.dram_tensor(
        "natg", [NGROUP, 128, GROUP * NCHUNK * DP], BF16, kind="ExternalInput"
    )
    w1rep_d = nc.dram_tensor("w1rep", [D, GROUP * D], BF16, kind="ExternalInput")
    brow_d = nc.dram_tensor("brow", [NGROUP, GROUP * D], BF16, kind="ExternalInput")
    maskm_d = nc.dram_tensor("maskm", [NUNIT // 4, 128, 64], F32, kind="ExternalInput")
    w1aug_d = nc.dram_tensor("w1aug", [D + 1, D], F32, kind="ExternalInput")
    identf_d = nc.dram_tensor("identf", [128, 128], F32, kind="ExternalInput")
    out_d = nc.dram_tensor("out", [BC, D], F32, kind="ExternalOutput")

    with tile.TileContext(nc) as tc, ExitStack() as ctx:
        const_pool = ctx.enter_context(tc.tile_pool(name="const", bufs=1))
        natp = ctx.enter_context(tc.tile_pool(name="natp", bufs=4))
        sigp = ctx.enter_context(tc.tile_pool(name="sigp", bufs=3))
        smp = ctx.enter_context(tc.tile_pool(name="smp", bufs=2))
        zpsum = ctx.enter_context(
            tc.tile_pool(name="zpsum", bufs=2, space=bass.MemorySpace.PSUM)
        )

        w1aug_sb = const_pool.tile([D + 1, D], F32)
        nc.scalar.dma_start(w1aug_sb[:], w1aug_d[:])
        identf_sb = const_pool.tile([128, 128], F32)
        nc.scalar.dma_start(identf_sb[:], identf_d[:])
        # persistent rhs tiles: [w1.T replicated 8x ; per-group bias row]
        rhs_tiles = []
        for k in (0, 1):
            t = const_pool.tile([D + 1, GROUP * D], BF16, name=f"rhs{k}", tag=f"rhs{k}")
            nc.scalar.dma_start(t[0:D, :], w1rep_d[:])
            rhs_tiles.append(t)
        # persistent seqt tiles; DMA fills rows 0:96 per group (96 = 12x8 so
        # the HWDGE splits it across 12 SDMA engines), ones row 96 is memset
        # once and persists
        seqt_tiles = []
        for k in range(3):
            t = const_pool.tile(
                [D + 1, GROUP * S], BF16, name=f"seqt{k}", tag=f"seqt{k}"
            )
            nc.vector.memset(t[D : D + 1, :], 1.0)
            seqt_tiles.append(t)
        # persistent zero-padded exp tiles: E(u4, q, j) at col u4*64 + q*17 + j*4
        exz_tiles = []
        for k in (0, 1):
            t = const_pool.tile([128, 256], BF16, name=f"exz{k}", tag=f"exz{k}")
            nc.vector.memset(t[:], 0.0)
            exz_tiles.append(t)

        group_sb = {}     # g -> (seqt tile, rhs tile, natg tile)
        tg_tiles = {}     # gg -> (scores [128,64] f32, mask [128,64] f32)
        exp_tiles = {}    # gg -> exz tile
        panel_pool = {}   # panel -> pooled [128, 97] f32

        for v in range(NUNIT + 4):
            # -------- produce stage for unit v --------
            zt_ps = zpsum.tile([128, 2048], F32, name="zt_ps")
            if v < NUNIT:
                u = v
                g, ug = divmod(u, 2)
                if ug == 0:
                    seqt_sb = seqt_tiles[g % 3]
                    nc.sync.dma_start(seqt_sb[0:D, :], seqt_d[g])
                    natt = natp.tile([128, GROUP * NCHUNK * DP], BF16, name="natt")
                    nc.sync.dma_start(natt[:], natg_d[g])
                    rhsg = rhs_tiles[g % 2]
                    nc.scalar.dma_start(rhsg[D : D + 1, :], brow_d[g : g + 1, :])
                    group_sb[g] = (seqt_sb, rhsg, natt)
                seqt_sb, rhsg, natt = group_sb[g]
                panel = u // UPP
                if u % UPP == 0:
                    pooled = smp.tile([128, DP], F32, name="pooled", tag="pooled")
                    panel_pool[panel] = pooled

                gg, u4 = divmod(u, 4)
                if u4 == 0:
                    sc2g = smp.tile([128, 64], F32, name="sc2g", tag="sc")
                    mk2g = smp.tile([128, 64], F32, name="mk2g", tag="mk")
                    nc.scalar.dma_start(mk2g[:], maskm_d[gg])
                    tg_tiles[gg] = (sc2g, mk2g)
                sc2g, mk2g = tg_tiles[gg]

                # 16 projection matmuls -> zT in PSUM
                for q in range(UNIT):
                    i = ug * UNIT + q      # batch within group
                    for j in range(NCHUNK):
                        nc.tensor.matmul(
                            zt_ps[:, q * 512 + j * D : q * 512 + (j + 1) * D],
                            seqt_sb[:, i * S + j * 128 : i * S + (j + 1) * 128],
                            rhsg[:, i * D : (i + 1) * D],
                            start=True,
                            stop=True,
                        )
                zt3 = zt_ps[:].rearrange("p (q c) -> p q c", q=UNIT)[
                    :, :, 0 : NCHUNK * D
                ]
                th = sigp.tile([128, UNIT * NCHUNK * D], BF16, name="th")
                nc.scalar.activation(
                    th[:].rearrange("p (q c) -> p q c", q=UNIT), zt3, Tanh, scale=0.5
                )
                # scores: bf16 2x add-tree then one 1x reduce -> fp32 [128, 16]
                th4 = th[:].rearrange("p (x e) -> p x e", e=D)
                ta = smp.tile([128, 16 * 48], BF16, name="ta", tag="ta")
                ta3 = ta[:].rearrange("p (x e) -> p x e", e=48)
                nc.vector.tensor_add(ta3, th4[:, :, 0:48], th4[:, :, 48:96])
                tb = smp.tile([128, 16 * 24], BF16, name="tb", tag="tb")
                tb3 = tb[:].rearrange("p (x e) -> p x e", e=24)
                nc.vector.tensor_add(tb3, ta3[:, :, 0:24], ta3[:, :, 24:48])
                nc.vector.reduce_sum(
                    sc2g[:, u4 * 16 : (u4 + 1) * 16], tb3, axis=AX
                )
                if u4 == 3:
                    sm2g = smp.tile([128, 64], F32, name="sm2g", tag="sm")
                    nc.vector.tensor_add(sm2g[:], sc2g[:], mk2g[:])
                    exz = exz_tiles[gg % 2]
                    exz_out = bass.AP(
                        tensor=exz.tensor,
                        offset=exz.offset,
                        ap=[[256, 128], [64, 4], [17, 4], [4, 4]],
                    )
                    nc.scalar.activation(exz_out, sm2g[:], Exp, scale=0.5)
                    exp_tiles[gg] = exz

            # -------- pool stage for unit v-4 --------
            if v >= 4:
                u = v - 4
                gg, u4 = divmod(u, 4)
                g, ug = divmod(u, 2)
                exz = exp_tiles[gg]
                natt = group_sb[g][2]
                for j in range(NCHUNK):
                    for q in range(UNIT):
                        i = ug * UNIT + q
                        base = u4 * 64 + q * 16 + j * 4
                        nc.tensor.matmul(
                            zt_ps[0:UNIT, 384 : 384 + DP],
                            exz[:, base : base + UNIT],
                            natt[:, (i * NCHUNK + j) * DP : (i * NCHUNK + j + 1) * DP],
                            start=(j == 0 and q == 0),
                            stop=(j == NCHUNK - 1 and q == UNIT - 1),
                        )
                panel = u // UPP
                pooled = panel_pool[panel]
                pu = u % UPP
                pstg = smp.tile([UNIT, DP], F32, name="pstg", tag="pstg", bufs=3)
                nc.vector.tensor_copy(pstg[:], zt_ps[0:UNIT, 384 : 384 + DP])
                nc.scalar.dma_start(
                    pooled[pu * UNIT : (pu + 1) * UNIT, 0:DP], pstg[:]
                )
                # -------- panel finalize --------
                if pu == UPP - 1:
                    rz = smp.tile([128, 1], F32, name="rz", tag="rz")
                    nc.vector.reciprocal(rz[:], pooled[:, D : D + 1])
                    pn = smp.tile([128, D], F32, name="pn", tag="pn")
                    nc.vector.tensor_scalar_mul(pn[:], pooled[:, 0:D], rz[:, 0:1])
                    # re-projection: transpose + augmented matmul in spare PSUM
                    pT_ps = zt_ps[0:D, 512 + 384 : 512 + 512]
                    nc.tensor.transpose(pT_ps, pn[:], identf_sb[:])
                    paug = smp.tile([D + 1, 128], F32, name="paug", tag="paug")
                    nc.vector.tensor_copy(paug[0:D, :], pT_ps)
                    nc.vector.memset(paug[D : D + 1, :], 1.0)
                    outp_ps = zt_ps[:, 2 * 512 + 384 : 2 * 512 + 384 + D]
                    nc.tensor.matmul(
                        outp_ps, paug[:], w1aug_sb[:], start=True, stop=True
                    )
                    osb = smp.tile([128, D], F32, name="osb", tag="osb")
                    nc.scalar.copy(osb[:], outp_ps)
                    nc.scalar.dma_start(
                        out_d[panel * PANEL : (panel + 1) * PANEL, :], osb[:]
                    )

    nc.compile()
    return nc


def prepare_in_maps(inputs: dict) -> list[dict]:
    seq = np.asarray(inputs["seq_item_embedding"], dtype=np.float32)
    tgt = np.asarray(inputs["target_item_embedding"], dtype=np.float32)
    mask = np.asarray(inputs["mask"])
    w1w = np.asarray(inputs["w1_weight"], dtype=np.float32)
    w1b = np.asarray(inputs["w1_bias"], dtype=np.float32)
    w2w = np.asarray(inputs["w2_weight"], dtype=np.float32)
    w2b = np.asarray(inputs["w2_bias"], dtype=np.float32)

    seq_bf = seq.astype(NP_BF16)
    bias_all = (tgt[:, 0, :] @ w2w.T + w2b + w1b).astype(np.float32)  # [B, D]
    m = mask[:, :S, 0]                                                # [B, S] bool

    w1rep = np.tile(np.ascontiguousarray(w1w.T), (1, GROUP)).astype(NP_BF16)
    w1aug = np.ascontiguousarray(
        np.concatenate([w1w.T, w1b[None, :]], axis=0).astype(np.float32)
    )
    identf = np.eye(128, dtype=np.float32)

    in_maps = []
    for c in range(N_CORES):
        sl = slice(c * BC, (c + 1) * BC)
        sc = seq_bf[sl]  # [BC, S, D]
        seqt = np.ascontiguousarray(
            sc.reshape(NGROUP, GROUP, S, D).transpose(0, 3, 1, 2)
        ).reshape(NGROUP, D, GROUP * S)
        ch = sc.reshape(NGROUP, GROUP, NCHUNK, 128, D)
        chp = np.concatenate(
            [ch, np.ones((NGROUP, GROUP, NCHUNK, 128, 1), dtype=NP_BF16)], axis=-1
        )
        natg = np.ascontiguousarray(chp.transpose(0, 3, 1, 2, 4)).reshape(
            NGROUP, 128, GROUP * NCHUNK * DP
        )
        # maskm[gg, p, b16*4 + j] = mask[b, j*128+p] ? -1e9 : -96
        mc = m[sl].reshape(NUNIT // 4, 16, NCHUNK, 128)
        maskm = np.where(mc, np.float32(-1e9), np.float32(-96.0)).transpose(
            0, 3, 1, 2
        ).reshape(NUNIT // 4, 128, 64)
        in_maps.append(
            {
                "seqt": seqt,
                "natg": natg,
                "w1rep": w1rep,
                "brow": np.ascontiguousarray(
                    bias_all[sl].astype(NP_BF16).reshape(NGROUP, GROUP * D)
                ),
                "maskm": np.ascontiguousarray(maskm),
                "w1aug": w1aug,
                "identf": identf,
            }
        )
    return in_maps


_CACHED_NC = None


def run(inputs: dict, trace: bool = False, tmpdir: str | None = None):
    global _CACHED_NC
    in_maps = prepare_in_maps(inputs)
    if _CACHED_NC is None:
        _CACHED_NC = build_program()
    res = run_bass_kernel_spmd(
        _CACHED_NC, in_maps, list(range(N_CORES)), trace=trace, tmpdir=tmpdir
    )
    out = np.concatenate([r["out"] for r in res.results], axis=0)
    return out, res


def kernel(**inputs) -> np.ndarray:
    out, _ = run(inputs, trace=False)
    return out
